# revision 32
# baseline (speedup 1.0000x reference)
"""MoE-LoRA layer kernel for Trainium2 (8 NeuronCores, data-parallel over tokens).

Computation (per reference):
  out = x @ W_base.T + b_base + scaling * sum_e combine[:,e] * (x @ A_e.T) @ B_e.T
  combine = renormalized top-2 softmax of router logits (= softmax over top-2 logits).

Sharding: 8192 tokens -> 1024 per core; all weights replicated. Layouts are
prepared host-side so device DMAs are contiguous (th = 512-token half):
  xt[p, th, kt, u]  = x[th*512+u, kt*128+p]        (bf16)
  x8[p, th, kf, u]  = x[th*512+u, (KB+kf)*128+p]   (fp8 copy, last KF k-tiles)
  wt[ot, p, kt, o]  = 64*W_base[ot*128+o, kt*128+p]   (bf16)
  w8[ot, p, kf, o]  = 64*W_base[ot*128+o, (KB+kf)*128+p]  (fp8)
  at[p, kt, er]     = A_all[er, kt*128+p]
  bt[er, o]         = 64*B_stack[e, o, r],  er = e*16+r
  rt[p, kt, e]      = W_router[e, kt*128+p]
  bias2[p, ot]      = b_base[ot*128+p]
Output: outt[ot, p, t] = out[t, ot*128+p]  (bf16).

The base GEMM is a split-K hybrid: KB k-tiles in bf16 (1 cycle/row) plus KF
k-tiles in fp8e4 DoubleRow (2 k-tiles/instruction, 2x rate). All base/LoRA
contributions carry a x64 scale so fp8 weights stay in e4m3 normal range;
the output activation applies scale=1/64 before the bias add. The window is
token-half-outer, with the softmax chain split so its matmul/transpose ops
never block the tensor queue mid-stream.
"""

import sys
import numpy as np
import ml_dtypes
from contextlib import ExitStack

try:
    import concourse.bass as bass
except ImportError:
    sys.path.insert(0, "/opt/trn_rl_repo")
    import concourse.bass as bass

import concourse.tile as tile
from concourse import bacc
from concourse import mybir
from concourse.bass import ts
from concourse.bass_utils import run_bass_kernel_spmd

F32 = mybir.dt.float32
BF16 = mybir.dt.bfloat16
FP8 = mybir.dt.float8e4
ALU = mybir.AluOpType
ACTF = mybir.ActivationFunctionType
AX = mybir.AxisListType
DR = mybir.MatmulPerfMode.DoubleRow
NPBF16 = ml_dtypes.bfloat16
NPFP8 = ml_dtypes.float8_e4m3

N_CORES = 8
D_IN = 4096
D_OUT = 4096
RANK = 16
NUM_EXPERTS = 8
ER = NUM_EXPERTS * RANK  # 128
TOP_K = 2
SCALING = 32.0 / RANK  # 2.0
KF = 8  # k-tiles of the base GEMM computed in fp8 DoubleRow (must be even)
WSCALE = 64.0  # power-of-2 scale keeping 64*W in e4m3 normal range


def build_nc(T=1024, KT=32, OT=32):
    """Build the per-core Bass kernel. T tokens, KT k-tiles (d_in=128*KT),
    OT out-tiles (d_out=128*OT). T must be a multiple of 512."""
    TH = T // 512  # token halves for 512-wide matmuls
    SPH = 512 // 128  # softmax subtiles per token half
    KB = KT - KF  # bf16 k-tiles
    nc = bacc.Bacc(None, target_bir_lowering=False, dynamic_dma_scratch_size=1024)

    xt = nc.dram_tensor("xt", [128, TH, KT, 512], BF16, kind="ExternalInput")
    x8t = nc.dram_tensor("x8t", [128, TH, KF, 512], FP8, kind="ExternalInput")
    wt = nc.dram_tensor("wt", [OT, 128, KB, 128], BF16, kind="ExternalInput")
    w8t = nc.dram_tensor("w8t", [OT, 128, KF, 128], FP8, kind="ExternalInput")
    at = nc.dram_tensor("at", [128, KB, ER], BF16, kind="ExternalInput")
    at8 = nc.dram_tensor("at8", [128, KF, ER], FP8, kind="ExternalInput")
    bt = nc.dram_tensor("bt", [ER, 128 * OT], BF16, kind="ExternalInput")
    rt = nc.dram_tensor("rt", [128, KT, NUM_EXPERTS], BF16, kind="ExternalInput")
    bias2 = nc.dram_tensor("bias2", [128, OT], F32, kind="ExternalInput")
    id2 = nc.dram_tensor("id2", [128, 128], F32, kind="ExternalInput")
    expand = nc.dram_tensor("expand", [NUM_EXPERTS, ER], BF16, kind="ExternalInput")
    outt = nc.dram_tensor("outt", [OT, 128, T], BF16, kind="ExternalOutput")
    INV = float(1.0 / WSCALE)

    with tile.TileContext(nc) as tc, ExitStack() as ctx:
        const = ctx.enter_context(tc.tile_pool(name="const", bufs=1))
        xpool = ctx.enter_context(tc.tile_pool(name="xp", bufs=1))
        wpool = ctx.enter_context(tc.tile_pool(name="wp", bufs=5))
        w8pool = ctx.enter_context(tc.tile_pool(name="w8p", bufs=5))
        btp = ctx.enter_context(tc.tile_pool(name="btp", bufs=4))
        hpool = ctx.enter_context(tc.tile_pool(name="hp", bufs=1))
        smt = ctx.enter_context(tc.tile_pool(name="smt", bufs=4))
        opool = ctx.enter_context(tc.tile_pool(name="op", bufs=4))
        pmain = ctx.enter_context(
            tc.tile_pool(name="pmain", bufs=max(3 * TH, 4), space="PSUM")
        )
        psmall = ctx.enter_context(tc.tile_pool(name="psm", bufs=2, space="PSUM"))
        E = NUM_EXPERTS

        # ---- PE warm-up: dummy matmuls on memset tiles run while the
        # startup DMAs stream, so the HAM clock gate reaches 8/8 before the
        # first real matmul instead of ~10us into the window ----
        warm_a = const.tile([128, 128], BF16)
        warm_x = const.tile([128, 512], BF16)
        nc.gpsimd.memset(warm_a, 0.0)
        nc.gpsimd.memset(warm_x, 0.0)

        # ---- DMAs: the x stream owns the scalar HWDGE queue (its own
        # descriptor generator); weights/consts flow on the sync queue ----
        at_s = wpool.tile([128, KB, 128], BF16, tag="w")
        w0_s = wpool.tile([128, KB, 128], BF16, tag="w")
        w1_s = wpool.tile([128, KB, 128], BF16, tag="w")
        x_s = xpool.tile([128, TH, KT, 512], BF16)
        x8_s = xpool.tile([128, TH, KF, 512], FP8)

        # x on the scalar queue, token-half-major to match the window order;
        # th0's head lands in fine grains (kt0 split in half, singles, pairs)
        # so the first window matmuls unblock ASAP, the rest as 4-ktile
        # chunks (4KB contiguous per partition). The sync queue is reserved
        # for the weight slivers the window consumes in lockstep with x.
        for th in range(TH):
            if th == 0:
                nc.scalar.dma_start(x_s[:, th, 0, 0:256], xt[:, th, 0, 0:256])
                nc.scalar.dma_start(x_s[:, th, 0, 256:512], xt[:, th, 0, 256:512])
                for kt in (1, 2, 3):
                    nc.scalar.dma_start(x_s[:, th, kt, :], xt[:, th, kt, :])
                for k0 in (4, 6, 8, 10, 12, 14):
                    nc.scalar.dma_start(
                        x_s[:, th, k0 : k0 + 2, :], xt[:, th, k0 : k0 + 2, :]
                    )
                k0 = 16
            else:
                k0 = 0
            while k0 < KT:
                k1 = min(k0 + 4, KT)
                nc.scalar.dma_start(x_s[:, th, k0:k1, :], xt[:, th, k0:k1, :])
                k0 = k1
            nc.scalar.dma_start(x8_s[:, th], x8t[:, th])

        # weights on sync queue, paced so early k-tiles don't steal the x
        # stream's HBM share: tiny head slivers, consts, then the rest
        rt_s = const.tile([128, KT, E], BF16)
        at8_s = const.tile([128, KF, ER], FP8)
        w08_s = const.tile([128, KF, 128], FP8)
        w18_s = const.tile([128, KF, 128], FP8)
        for sl in (slice(0, 1), slice(1, 2), slice(2, 8)):
            nc.sync.dma_start(at_s[:, sl, :], at[:, sl, :])
            nc.sync.dma_start(w0_s[:, sl, :], wt[0, :, sl, :])
            nc.sync.dma_start(w1_s[:, sl, :], wt[1, :, sl, :])
        nc.sync.dma_start(rt_s, rt[:])

        # consts ride the vector/gpsimd DGE queues: keeps them off the busy
        # sync queue AND spins those rings up early, so the output drain at
        # the very end can fan across four warm queues
        id_s = const.tile([128, 128], F32)
        nc.gpsimd.dma_start(id_s, id2[:])
        bias_s = const.tile([128, OT], F32)
        nc.gpsimd.dma_start(bias_s, bias2[:])
        exp_s = const.tile([E, ER], BF16)
        nc.gpsimd.dma_start(exp_s, expand[:])
        b0_s = const.tile([ER, 128], BF16)
        nc.gpsimd.dma_start(b0_s, bt[:, 0:128])
        b1_s = const.tile([ER, 128], BF16)
        nc.gpsimd.dma_start(b1_s, bt[:, 128:256])

        mid = (8 + KB) // 2
        for sl in (slice(8, mid), slice(mid, KB)):
            nc.sync.dma_start(at_s[:, sl, :], at[:, sl, :])
            nc.sync.dma_start(w0_s[:, sl, :], wt[0, :, sl, :])
            nc.sync.dma_start(w1_s[:, sl, :], wt[1, :, sl, :])
        nc.sync.dma_start(at8_s, at8[:])
        nc.sync.dma_start(w08_s, w8t[0, :, :, :])
        nc.sync.dma_start(w18_s, w8t[1, :, :, :])

        # ---- window: A-proj + base(ot=0) share the x stream in 256-col
        # half-matmuls (each stationary feeds 2 back-to-back matmuls, which
        # sustains the steady-state issue rate; one-matmul-per-stationary
        # degrades ~40% on the weight-buffer recycle path). The router runs
        # as 4-way column-tiled bundles at the end of each half: 4 k-tiles
        # compute concurrently in distinct 32-column PE groups. ----
        # creation order drives the pool's slot rotation: the 6 tiles live
        # at once (ph0, plT0, po0[0..1], po1[0..1]) take the 6 slots; ph1 and
        # plT1 then recycle ph0/plT0's slots, which free after th0's copies.
        ph = [pmain.tile([128, 512], F32, tag="pm", name="ph0")]
        plT = [pmain.tile([128, 512], F32, tag="pm", name="plT0")]
        po0 = [
            pmain.tile([128, 512], F32, tag="pm", name=f"po0{i}") for i in range(TH)
        ]
        po1 = [
            pmain.tile([128, 512], F32, tag="pm", name=f"po1{i}") for i in range(TH)
        ]
        if TH > 1:
            ph.append(pmain.tile([128, 512], F32, tag="pm", name="ph1"))
            plT.append(pmain.tile([128, 512], F32, tag="pm", name="plT1"))
        RG = KT // 4  # k-tiles per router column-group

        h_s = hpool.tile([128, T], BF16)
        hw_r = h_s  # weighted in place; rhs of the B matmuls
        # router logits land in rows {32g..32g+7}; the other rows are never
        # written, so zero them once — the full-width transpose reads them
        # and stale NaN bit patterns would poison whole output columns
        lT = [hpool.tile([128, 512], F32, name=f"lT{i}") for i in range(TH)]
        for t_ in lT:
            nc.gpsimd.memset(t_, 0.0)
        l_t = {}

        # warm-up matmuls land in ph[0]; the real A-projection's start=True
        # overwrites whatever they left there
        for _ in range(8):
            nc.tensor.matmul(
                ph[0], warm_a, warm_x, start=True, stop=True, skip_group_check=True
            )

        def softmax_a(s_i):
            """transpose this 128-token chunk of the 4-group router PSUM out
            to token-major and reduce the 4 column groups."""
            th, q = divmod(s_i, SPH)
            ptl = psmall.tile([128, 128], F32, tag="ps", name="ptl")
            nc.tensor.transpose(ptl, lT[th][:, ts(q, 128)], id_s)
            pls = smt.tile([128, 128], F32, name="pls")
            nc.vector.tensor_copy(pls, ptl)
            t1 = smt.tile([128, E], F32, name="t1")
            nc.vector.tensor_tensor(t1, pls[:, 0:E], pls[:, 32 : 32 + E], op=ALU.add)
            t2 = smt.tile([128, E], F32, name="t2")
            nc.vector.tensor_tensor(t2, pls[:, 64 : 64 + E], pls[:, 96 : 96 + E], op=ALU.add)
            l = smt.tile([128, E], F32, name="l")
            nc.vector.tensor_tensor(l, t1, t2, op=ALU.add)
            l_t[s_i] = l

        def softmax_b(s_i):
            """combine weights for tokens [s_i*128, (s_i+1)*128), weighted
            into hw_r in place. The DVE chain runs behind whatever matmul
            stream precedes this in program order."""
            l = l_t.pop(s_i)
            m1 = smt.tile([128, 1], F32)
            nc.vector.reduce_max(m1, l, axis=AX.X)
            lm = smt.tile([128, E], F32)  # logits - max  (<= 0, ==0 at argmax)
            nc.vector.tensor_scalar(lm, l, m1, None, op0=ALU.subtract)
            isz = smt.tile([128, E], F32)
            nc.vector.tensor_scalar(isz, lm, 0.0, None, op0=ALU.is_equal)
            pen = smt.tile([128, E], F32)
            nc.vector.tensor_scalar(pen, isz, -1e30, None, op0=ALU.mult)
            msk = smt.tile([128, E], F32)
            nc.vector.tensor_tensor(msk, lm, pen, op=ALU.add)
            m2 = smt.tile([128, 1], F32)  # second max, relative to m1
            nc.vector.reduce_max(m2, msk, axis=AX.X)
            e_t = smt.tile([128, E], F32)
            nc.scalar.activation(e_t, lm, ACTF.Exp)
            e2 = smt.tile([128, 1], F32)
            nc.scalar.activation(e2, m2, ACTF.Exp)
            den = smt.tile([128, 1], F32)
            nc.vector.tensor_scalar(den, e2, 1.0, None, op0=ALU.add)
            inv = smt.tile([128, 1], F32)
            nc.vector.reciprocal(inv, den)
            ge = smt.tile([128, E], F32)  # top-2 membership mask
            nc.vector.tensor_scalar(ge, lm, m2, None, op0=ALU.is_ge)
            cmb = smt.tile([128, E], F32)
            nc.vector.tensor_tensor(cmb, e_t, ge, op=ALU.mult)
            cmb2 = smt.tile([128, E], F32)
            nc.vector.tensor_scalar(cmb2, cmb, inv, None, op0=ALU.mult)
            pt = psmall.tile([E, 128], F32, tag="ps", name="pt")
            nc.tensor.transpose(pt, cmb2, id_s)
            ct = smt.tile([E, 128], BF16)
            nc.vector.tensor_copy(ct, pt)
            pc = psmall.tile([128, 128], F32, tag="ps", name="pc")
            nc.tensor.matmul(pc, exp_s, ct, start=True, stop=True)
            nc.vector.tensor_tensor(
                hw_r[:, ts(s_i, 128)], h_s[:, ts(s_i, 128)], pc, op=ALU.mult
            )

        def emit_window_half(th):
            # The start=True has_written clear covers ALL columns of the
            # bank on the partitions the matmul writes, so kt0 runs as one
            # full-width matmul per stream; later k-tiles run as 256-col
            # halves (2 matmuls per stationary sustains the steady-state
            # issue rate; one-per-stationary degrades ~40%).
            xc0 = x_s[:, th, 0, :]
            nc.tensor.matmul(ph[th], at_s[:, 0, :], xc0, start=True, stop=False)
            nc.tensor.matmul(po0[th], w0_s[:, 0, :], xc0, start=True, stop=False)
            nc.tensor.matmul(po1[th], w1_s[:, 0, :], xc0, start=True, stop=False)
            # th0's head runs full-width: ~305ns/matmul of deliberate pacing
            # while the x stream is still ramping — slower lockstep with the
            # DMA beats lumpy multi-us stalls that re-throttle the HAM clock
            ramp = 10 if th == 0 else 1
            for kt in range(1, ramp):
                xc = x_s[:, th, kt, :]
                nc.tensor.matmul(ph[th], at_s[:, kt, :], xc, start=False, stop=False)
                nc.tensor.matmul(po0[th], w0_s[:, kt, :], xc, start=False, stop=False)
                nc.tensor.matmul(po1[th], w1_s[:, kt, :], xc, start=False, stop=False)
            for kt in range(ramp, KB):
                xc = x_s[:, th, kt, :]
                for q in range(2):
                    nc.tensor.matmul(
                        ph[th][:, ts(q, 256)], at_s[:, kt, :], xc[:, ts(q, 256)],
                        start=False, stop=False,
                    )
                for q in range(2):
                    nc.tensor.matmul(
                        po0[th][:, ts(q, 256)], w0_s[:, kt, :], xc[:, ts(q, 256)],
                        start=False, stop=False,
                    )
                for q in range(2):
                    nc.tensor.matmul(
                        po1[th][:, ts(q, 256)], w1_s[:, kt, :], xc[:, ts(q, 256)],
                        start=False, stop=False,
                    )
            # fp8 tail: A and base0/1 as DoubleRow pairs (256-col halves
            # keep the 2-matmuls-per-stationary cadence)
            for j in range(KF // 2):
                kf = 2 * j
                last = j == KF // 2 - 1
                for q in range(2):
                    nc.tensor.matmul(
                        ph[th][:, ts(q, 256)],
                        at8_s[:, kf : kf + 2, :],
                        x8_s[:, th, kf : kf + 2, ts(q, 256)],
                        start=False, stop=last, perf_mode=DR,
                    )
                for q in range(2):
                    nc.tensor.matmul(
                        po0[th][:, ts(q, 256)],
                        w08_s[:, kf : kf + 2, :],
                        x8_s[:, th, kf : kf + 2, ts(q, 256)],
                        start=False, stop=False, perf_mode=DR,
                    )
                for q in range(2):
                    nc.tensor.matmul(
                        po1[th][:, ts(q, 256)],
                        w18_s[:, kf : kf + 2, :],
                        x8_s[:, th, kf : kf + 2, ts(q, 256)],
                        start=False, stop=False, perf_mode=DR,
                    )
            # router: bundles of 4 column-group-concurrent matmuls; the
            # full x half is resident by now, so k-tiles split across
            # groups. Each group's i=0 matmul carries start=True — the
            # has_written clear only touches that group's partitions, so
            # the concurrent clears are disjoint.
            for i in range(RG):
                last = i == RG - 1
                for g in range(4):
                    kt = g * RG + i
                    nc.tensor.matmul(
                        plT[th][g * 32 : g * 32 + E, :],
                        rt_s[:, kt, :],
                        x_s[:, th, kt, :],
                        start=(i == 0), stop=last, tile_position=(0, g * 32),
                    )
            nc.vector.tensor_copy(h_s[:, ts(th, 512)], ph[th])
            # gather only the 4 written 8-row group slices out of PSUM (the
            # unwritten psum partitions can hold stale NaNs)
            for g in range(4):
                nc.vector.tensor_copy(
                    lT[th][g * 32 : g * 32 + E, :], plT[th][g * 32 : g * 32 + E, :]
                )
            for s_i in range(th * SPH, (th + 1) * SPH):
                softmax_a(s_i)

        emit_window_half(0)
        if TH > 1:
            emit_window_half(1)
        for s_i in range(0, SPH):
            softmax_b(s_i)
        if TH > 1:
            for s_i in range(SPH, 2 * SPH):
                softmax_b(s_i)

        def load_w(ot, split=False):
            w_s = wpool.tile([128, KB, 128], BF16, tag="w")
            if split:
                # first half unblocks the o-tile's first matmuls; the second
                # trails in behind them, keeping those bytes out of the
                # DMA-bound window period
                half = KB // 2
                nc.sync.dma_start(w_s[:, 0:half, :], wt[ot, :, 0:half, :])
                nc.sync.dma_start(w_s[:, half:KB, :], wt[ot, :, half:KB, :])
            else:
                nc.sync.dma_start(w_s, wt[ot, :, :, :])
            w8_s = w8pool.tile([128, KF, 128], FP8, tag="w8")
            nc.sync.dma_start(w8_s, w8t[ot, :, :, :])
            b_sl = btp.tile([ER, 128], BF16)
            nc.sync.dma_start(b_sl, bt[:, ts(ot, 128)])
            return w_s, w8_s, b_sl

        def emit_base(ot, w_s, w8_s, b_sl=None):
            # kt outer / th inner: consecutive matmuls share the stationary
            # weight tile; the last KF k-tiles run as fp8 DoubleRow pairs.
            # When b_sl is given (hw_r already final), the LoRA B matmul
            # leads the accumulation group instead of trailing it.
            pos = [
                pmain.tile([128, 512], F32, tag="pm", name=f"po_{ot}_{th}")
                for th in range(TH)
            ]
            if b_sl is not None:
                for th in range(TH):
                    nc.tensor.matmul(
                        pos[th], b_sl, hw_r[:, ts(th, 512)], start=True, stop=False
                    )
            for kt in range(KB):
                for th in range(TH):
                    nc.tensor.matmul(
                        pos[th],
                        w_s[:, kt, :],
                        x_s[:, th, kt, :],
                        start=(kt == 0 and b_sl is None),
                        stop=False,
                    )
            for kf in range(0, KF, 2):
                last = b_sl is not None and kf == KF - 2
                for th in range(TH):
                    nc.tensor.matmul(
                        pos[th],
                        w8_s[:, kf : kf + 2, :],
                        x8_s[:, th, kf : kf + 2, :],
                        start=False,
                        stop=last,
                        perf_mode=DR,
                    )
            return pos

        def emit_tail(ot, pos, b_sl):
            for th in range(TH):
                if b_sl is not None:
                    nc.tensor.matmul(
                        pos[th], b_sl, hw_r[:, ts(th, 512)], start=False, stop=True
                    )
                o_t = opool.tile([128, 512], BF16, tag="o_t", name=f"ot_{ot}_{th}")
                nc.scalar.activation(
                    o_t, pos[th], ACTF.Identity, bias=bias_s[:, ot : ot + 1], scale=INV
                )
                nc.sync.dma_start(outt[ot, :, ts(th, 512)], o_t)

        def emit_tail_last(ot, pos):
            # drain the final o-tile in 256-token chunks, alternating the
            # scalar and vector engines, with the output DMAs fanned across
            # four pre-warmed DGE queues so no single ring serializes them
            dqs = [nc.sync, nc.scalar, nc.gpsimd, nc.sync]
            for th in range(TH):
                for q in range(2):
                    o_q = opool.tile(
                        [128, 256], BF16, tag="o_t", name=f"oq_{th}_{q}"
                    )
                    src = pos[th][:, ts(q, 256)]
                    if q == 0:
                        nc.scalar.activation(
                            o_q, src, ACTF.Identity,
                            bias=bias_s[:, ot : ot + 1], scale=INV,
                        )
                    else:
                        nc.vector.tensor_scalar(
                            o_q, src, INV, bias_s[:, ot : ot + 1],
                            op0=ALU.mult, op1=ALU.add,
                        )
                    dqs[th * 2 + q].dma_start(outt[ot, :, ts(th * 2 + q, 256)], o_q)

        # ---- first steady pair (ot=2,3): its k-loops are independent of
        # hw_r, so they cover the softmax_b latency right after the window;
        # the LoRA B matmuls sit late in each group instead of leading it.
        def emit_pair_first(a, b):
            w_a, w8_a, b_a = load_w(a, split=True)
            w_b, w8_b, b_b = load_w(b)
            pos_a = [
                pmain.tile([128, 512], F32, tag="pm", name=f"po_{a}_{th}")
                for th in range(TH)
            ]
            pos_b = [
                pmain.tile([128, 512], F32, tag="pm", name=f"po_{b}_{th}")
                for th in range(TH)
            ]
            for kt in range(KB):
                for th in range(TH):
                    nc.tensor.matmul(
                        pos_a[th], w_a[:, kt, :], x_s[:, th, kt, :],
                        start=(kt == 0), stop=False,
                    )
            for kf in range(0, KF, 2):
                for th in range(TH):
                    nc.tensor.matmul(
                        pos_a[th], w8_a[:, kf : kf + 2, :],
                        x8_s[:, th, kf : kf + 2, :],
                        start=False, stop=False, perf_mode=DR,
                    )
            for kf in range(0, KF, 2):
                for th in range(TH):
                    nc.tensor.matmul(
                        pos_b[th], w8_b[:, kf : kf + 2, :],
                        x8_s[:, th, kf : kf + 2, :],
                        start=(kf == 0), stop=False, perf_mode=DR,
                    )
            for th in range(TH):
                nc.tensor.matmul(
                    pos_a[th], b_a, hw_r[:, ts(th, 512)], start=False, stop=True
                )
            for th in range(TH):
                nc.tensor.matmul(
                    pos_b[th], b_b, hw_r[:, ts(th, 512)], start=False, stop=False
                )
            for kt in range(KB):
                for th in range(TH):
                    nc.tensor.matmul(
                        pos_b[th], w_b[:, kt, :], x_s[:, th, kt, :],
                        start=False, stop=(kt == KB - 1),
                    )
            return pos_a, pos_b

        pos_a2, pos_b2 = emit_pair_first(2, 3)
        emit_tail(2, pos_a2, None)
        emit_tail(3, pos_b2, None)

        # ---- ot=0/1 LoRA terms accumulated into the held PSUM groups ----
        for po_, b_s_, oti in ((po0, b0_s, 0), (po1, b1_s, 1)):
            for th in range(TH):
                for q in range(2):
                    nc.tensor.matmul(
                        po_[th][:, ts(q, 256)], b_s_, hw_r[:, ts(th * 2 + q, 256)],
                        start=False, stop=True,
                    )
                o_t = opool.tile([128, 512], BF16, name=f"oo{oti}_{th}", tag="o_t")
                nc.scalar.activation(
                    o_t, po_[th], ACTF.Identity, bias=bias_s[:, oti : oti + 1],
                    scale=INV,
                )
                nc.sync.dma_start(outt[oti, :, ts(th, 512)], o_t)

        # ---- remaining o-tiles, processed in pairs with the two fp8
        # DoubleRow sections adjacent: the PE pays the fp8 stationary
        # double-load once per pair instead of once per o-tile. hw_r is
        # final here, so the LoRA B matmul can sit anywhere in each group.
        def emit_pair(a, b):
            w_a, w8_a, b_a = load_w(a)
            w_b, w8_b, b_b = load_w(b)
            pos_a = [
                pmain.tile([128, 512], F32, tag="pm", name=f"po_{a}_{th}")
                for th in range(TH)
            ]
            pos_b = [
                pmain.tile([128, 512], F32, tag="pm", name=f"po_{b}_{th}")
                for th in range(TH)
            ]
            for th in range(TH):
                nc.tensor.matmul(
                    pos_a[th], b_a, hw_r[:, ts(th, 512)], start=True, stop=False
                )
            for kt in range(KB):
                for th in range(TH):
                    nc.tensor.matmul(
                        pos_a[th], w_a[:, kt, :], x_s[:, th, kt, :],
                        start=False, stop=False,
                    )
            for kf in range(0, KF, 2):
                for th in range(TH):
                    nc.tensor.matmul(
                        pos_a[th], w8_a[:, kf : kf + 2, :],
                        x8_s[:, th, kf : kf + 2, :],
                        start=False, stop=(kf == KF - 2), perf_mode=DR,
                    )
            for kf in range(0, KF, 2):
                for th in range(TH):
                    nc.tensor.matmul(
                        pos_b[th], w8_b[:, kf : kf + 2, :],
                        x8_s[:, th, kf : kf + 2, :],
                        start=(kf == 0), stop=False, perf_mode=DR,
                    )
            for th in range(TH):
                nc.tensor.matmul(
                    pos_b[th], b_b, hw_r[:, ts(th, 512)], start=False, stop=False
                )
            for kt in range(KB):
                for th in range(TH):
                    nc.tensor.matmul(
                        pos_b[th], w_b[:, kt, :], x_s[:, th, kt, :],
                        start=False, stop=(kt == KB - 1),
                    )
            return pos_a, pos_b

        ot = 4
        while ot < OT:
            if ot + 1 < OT:
                pos_a, pos_b = emit_pair(ot, ot + 1)
                emit_tail(ot, pos_a, None)
                if ot + 1 == OT - 1:
                    emit_tail_last(ot + 1, pos_b)
                else:
                    emit_tail(ot + 1, pos_b, None)
                ot += 2
            else:
                w_s, w8_s, b_sl = load_w(ot)
                pos = emit_base(ot, w_s, w8_s, b_sl)
                emit_tail_last(ot, pos)
                ot += 1

    nc.compile()
    return nc


def prep_shared(W_base, b_base, W_router, A_stack, B_stack, KT=32, OT=32):
    """Host-side layout prep for the replicated weights."""
    D = KT * 128
    O = OT * 128
    KB = KT - KF
    W_base = np.asarray(W_base, dtype=np.float32) * np.float32(WSCALE)
    w4 = W_base.reshape(OT, 128, KT, 128).transpose(0, 3, 2, 1)  # [ot, p, kt, o]
    wt = np.ascontiguousarray(w4[:, :, :KB, :]).astype(NPBF16)
    w8 = np.ascontiguousarray(w4[:, :, KB:, :]).astype(NPFP8)
    # A carries the same x64 scale (compensated via the expand matrix)
    A_all = np.asarray(A_stack, dtype=np.float32).reshape(ER, D) * np.float32(WSCALE)
    a3 = A_all.reshape(ER, KT, 128).transpose(2, 1, 0)  # [p, kt, er]
    at = np.ascontiguousarray(a3[:, :KB, :]).astype(NPBF16)
    at8 = np.ascontiguousarray(a3[:, KB:, :]).astype(NPFP8)
    bt = np.ascontiguousarray(
        np.asarray(B_stack, dtype=np.float32).transpose(0, 2, 1).reshape(ER, O)
        * np.float32(WSCALE)
    ).astype(NPBF16)
    rtT = np.asarray(W_router, dtype=np.float32).T  # [D, E]
    rt = np.ascontiguousarray(
        rtT.reshape(KT, 128, NUM_EXPERTS).transpose(1, 0, 2)
    ).astype(NPBF16)
    bias2 = np.ascontiguousarray(np.asarray(b_base, dtype=np.float32).reshape(OT, 128).T)
    id2 = np.eye(128, dtype=np.float32)
    expand = np.repeat(
        np.eye(NUM_EXPERTS, dtype=np.float32) * np.float32(SCALING / WSCALE),
        RANK, axis=1,
    ).astype(NPBF16)
    return dict(
        wt=wt, w8t=w8, at=at, at8=at8, bt=bt, rt=rt, bias2=bias2, id2=id2,
        expand=expand,
    )


def make_in_maps(x, W_base, b_base, W_router, A_stack, B_stack, T=1024, KT=32, OT=32):
    shared = prep_shared(W_base, b_base, W_router, A_stack, B_stack, KT, OT)
    KB = KT - KF
    TH = T // 512
    xf = np.asarray(x, dtype=np.float32).reshape(-1, D_IN)
    in_maps = []
    for c in range(N_CORES):
        x_c = xf[c * T : (c + 1) * T]  # [T, D]
        # [p, th, kt, u]
        x4 = x_c.reshape(TH, 512, KT, 128).transpose(3, 0, 2, 1)
        xt = np.ascontiguousarray(x4).astype(NPBF16)
        x8 = np.ascontiguousarray(x4[:, :, KB:, :]).astype(NPFP8)
        m = dict(shared)
        m["xt"] = xt
        m["x8t"] = x8
        in_maps.append(m)
    return in_maps


_NC_CACHE = {}


def _get_nc(T, KT, OT):
    key = (T, KT, OT)
    if key not in _NC_CACHE:
        _NC_CACHE[key] = build_nc(T, KT, OT)
    return _NC_CACHE[key]


def kernel(x, W_base, b_base, W_router, A_stack, B_stack):
    x = np.asarray(x, dtype=np.float32)
    orig_shape = x.shape
    N = x.reshape(-1, D_IN).shape[0]
    T = N // N_CORES
    KT = D_IN // 128
    OT = D_OUT // 128

    nc = _get_nc(T, KT, OT)
    in_maps = make_in_maps(x, W_base, b_base, W_router, A_stack, B_stack, T, KT, OT)

    res = run_bass_kernel_spmd(nc, in_maps, core_ids=list(range(N_CORES)))
    out = np.empty((N, D_OUT), dtype=np.float32)
    for c in range(N_CORES):
        outt = res.results[c]["outt"]  # [OT, 128, T] bf16
        out[c * T : (c + 1) * T] = (
            outt.astype(np.float32).transpose(2, 0, 1).reshape(T, D_OUT)
        )
    return out.reshape(orig_shape[:-1] + (D_OUT,))

